# revision 7
# baseline (speedup 1.0000x reference)
# Trainium2 Bass kernel: 2:4 structured activation pruning + Linear.
#
#   out = magnitude_prune_2of4(x.reshape(-1, 4096)) @ weight.T
#
# Sharding: data-parallel over the flattened token dim (16384 tokens ->
# 2048/core across 8 cores); weight replicated (host-transposed + bf16 so
# the contraction dim lands on SBUF partitions). No collectives.
#
# v2 pipeline (PE does ONLY matmuls; transposes moved to the DMA xbar):
#   DMA x (f32) -> DVE |x| pairwise max/min (custom ops, exact f32)
#   -> DVE compact tree -> per-group-of-4 2nd-max threshold (exact f32)
#   -> DVE prune-select writing bf16 -> DMA-xbar transpose (SBUF->SBUF,
#   scalar HWDGE queue) -> PE bf16 matmuls (FWL weight loads) accumulating
#   over 32 d-chunks -> ACT PSUM->SBUF copy -> DMA out (f32).
import numpy as np

N_CORES = 8
BS, SEQ, D = 4, 4096, 4096
OUTF = 1024
TOK_TOTAL = BS * SEQ
TOK = TOK_TOTAL // N_CORES      # 2048 tokens per core
P = 128                         # SBUF partitions
NT = TOK // P                   # 16 token tiles per core
HALF = D // 2                   # 2048: free-dim half width
NCH = D // P                    # 32 d-chunks of 128
NCH_H = NCH // 2                # 16 d-chunks per half

_compiled = None
_custom_ops = None


def _register_custom_dve():
    # Fused DVE ops (registered into the runtime op table; compiled into the
    # per-NEFF DVE table): pairwise abs-max/abs-min, and the pruning select
    # out = |x| >= thr ? x : 0. Halves DVE work vs stock-op sequences.
    global _custom_ops
    if _custom_ops is not None:
        return _custom_ops
    from concourse import dve_ops as D
    from concourse.dve_spec import Spec, Src0, Src1, Zero, maxx, minn, select, lower
    from concourse.dve_uop import DveOpSpec

    def mk(name, body, reference):
        spec = Spec(body=body, reference=reference)
        shas = {}
        for ver in ("v3", "v4"):
            try:
                u = lower(spec, ver=ver)
                shas[ver] = DveOpSpec(name=name, opcode=1, uops=u,
                                      rd1_en=True).sha(ver)
            except Exception:
                if ver == "v3":
                    raise
        return D.DveOp(name=name, spec=spec, subdim=False, uops_sha=shas)

    absa = maxx(Src0, Zero - Src0)
    absb = maxx(Src1, Zero - Src1)
    ops = (
        mk("ABS_MAX2_ANT", maxx(absa, absb),
           lambda in0, in1: np.maximum(np.abs(in0), np.abs(in1))),
        mk("ABS_MIN2_ANT", minn(absa, absb),
           lambda in0, in1: np.minimum(np.abs(in0), np.abs(in1))),
        mk("PRUNE24_ANT", select(maxx(Src0, Zero - Src0) >= Src1, Src0, Zero),
           lambda in0, in1: np.where(np.abs(in0) >= in1, in0, 0.0)),
    )
    for op in ops:
        if op.name not in D._SUB_OPCODE_FOR_NAME:
            D.OPS.append(op)
            D.CUSTOM_DVE_SPECS[op.name] = op.spec
            D._SUB_OPCODE_FOR_NAME[op.name] = (
                D._CUSTOM_DVE_ROW_BASE + len(D._SUB_OPCODE_FOR_NAME))
    _custom_ops = ops
    return ops


def _build():
    import concourse.tile as tile
    import concourse.mybir as mybir
    from concourse import bacc

    ABS_MAX2, ABS_MIN2, PRUNE24 = _register_custom_dve()
    f32 = mybir.dt.float32
    bf16 = mybir.dt.bfloat16
    Alu = mybir.AluOpType

    nc = bacc.Bacc("TRN2", target_bir_lowering=False, debug=False,
                   num_devices=N_CORES)
    xs_ap = nc.dram_tensor("xs", [TOK, D], f32, kind="ExternalInput").ap()
    wt_ap = nc.dram_tensor("wt", [D, OUTF], bf16, kind="ExternalInput").ap()
    o_ap = nc.dram_tensor("o", [TOK, OUTF], f32, kind="ExternalOutput").ap()

    with tile.TileContext(nc) as tc:
        with tc.tile_pool(name="wpool", bufs=1) as wpool, \
             tc.tile_pool(name="xin", bufs=2) as xin, \
             tc.tile_pool(name="mwork", bufs=2) as mwork, \
             tc.tile_pool(name="twork", bufs=2) as twork, \
             tc.tile_pool(name="spool", bufs=2) as spool, \
             tc.tile_pool(name="xtp", bufs=2) as xtp, \
             tc.tile_pool(name="outp", bufs=2) as outp, \
             tc.tile_pool(name="pso", bufs=4, space="PSUM") as pso:

            # weight.T resident in SBUF: [d-in-chunk partitions, chunk, outf],
            # bf16, split in two tiles so the first 16 chunk-matmuls only
            # wait on the first 4MB.  One 3D DMA each on the gpsimd queue so
            # the transfer doesn't serialize ahead of the x-tile loads.
            w_halves = []
            for hw in range(2):
                w_h = wpool.tile([P, NCH_H, OUTF], bf16, tag=f"w{hw}")
                src = wt_ap[hw * (D // 2):(hw + 1) * (D // 2), :]
                nc.gpsimd.dma_start(
                    out=w_h, in_=src.rearrange("(c p) o -> p c o", p=P))
                w_halves.append(w_h)

            def process_span(i, xT, h):
                # prune x[i-tile, h-half] and deposit the transposed bf16
                # chunks into xT[:, h*16:(h+1)*16, :] via the DMA xbar.
                xh = xin.tile([P, HALF], f32, tag="xh")
                nc.sync.dma_start(out=xh, in_=xs_ap[i * P:(i + 1) * P,
                                                    h * HALF:(h + 1) * HALF])
                # pairwise tree: thr = 2nd-largest |x| per group of 4
                x2 = xh.rearrange("p (g two) -> p g two", two=2)
                mx = mwork.tile([P, HALF // 2], f32, tag="mx")
                mn = mwork.tile([P, HALF // 2], f32, tag="mn")
                nc.vector._custom_dve(ABS_MAX2, out=mx,
                                      in0=x2[:, :, 0], in1=x2[:, :, 1])
                nc.vector._custom_dve(ABS_MIN2, out=mn,
                                      in0=x2[:, :, 0], in1=x2[:, :, 1])
                # compact: 2nd-max = max(min of pair-maxes, max of pair-mins)
                mx2 = mx.rearrange("p (g two) -> p g two", two=2)
                mn2 = mn.rearrange("p (g two) -> p g two", two=2)
                mm = twork.tile([P, HALF // 4], f32, tag="mm")
                nm = twork.tile([P, HALF // 4], f32, tag="nm")
                nc.vector.tensor_tensor(mm, mx2[:, :, 0], mx2[:, :, 1], Alu.min)
                nc.vector.tensor_tensor(nm, mn2[:, :, 0], mn2[:, :, 1], Alu.max)
                thr = mm
                nc.vector.tensor_tensor(thr, mm, nm, Alu.max)
                # prune: xspr = |x| >= thr ? x : 0, cast to bf16 on write
                thr_b = thr.unsqueeze(2).broadcast_to([P, HALF // 4, 4])
                xspr = spool.tile([P, HALF], bf16, tag="xspr")
                nc.vector._custom_dve(
                    PRUNE24,
                    out=xspr.rearrange("p (g four) -> p g four", four=4),
                    in0=xh.rearrange("p (g four) -> p g four", four=4),
                    in1=thr_b)
                # SBUF->SBUF 128x128 transposes via the DMA xbar:
                # out[p, c, t] = xspr[t, 128c + p]
                nc.scalar.dma_start(out=xT[h], in_=xspr, transpose=True)

            for i in range(NT):
                # bf16 transposed pruned activations, [d, chunk, tok];
                # one tile per half so matmuls of half 0 can start while
                # half 1 is still being pruned/transposed.
                xT0 = xtp.tile([P, NCH_H, P], bf16, tag="xt0")
                xT1 = xtp.tile([P, NCH_H, P], bf16, tag="xt1")
                xT = [xT0, xT1]
                for h in range(2):
                    process_span(i, xT, h)
                # matmul: psum[tok, outf-half] += xT[h][c].T @ wT[h][c]
                for n in range(2):
                    pout = pso.tile([P, OUTF // 2], f32)
                    for h in range(2):
                        for c in range(NCH_H):
                            nc.tensor.matmul(
                                pout,
                                xT[h][:, c, :],
                                w_halves[h][:, c, n * 512:(n + 1) * 512],
                                start=(h == 0 and c == 0),
                                stop=(h == 1 and c == NCH_H - 1))
                    osb = outp.tile([P, OUTF // 2], f32)
                    nc.scalar.copy(osb, pout)
                    nc.gpsimd.dma_start(
                        out=o_ap[i * P:(i + 1) * P, n * 512:(n + 1) * 512],
                        in_=osb)
    nc.compile()
    return nc


def _get_compiled():
    global _compiled
    if _compiled is None:
        _compiled = _build()
    return _compiled


def _fix_ties(x_flat):
    # The device keeps elements with |x| >= (2nd-largest |x| of the group).
    # On an exact fp32 tie |2nd|==|3rd| that keeps 3 elements, while the
    # reference (top_k, stable) keeps the lower-indexed 2. Pre-zero the
    # reference-dropped elements of tied groups so the device agrees; the
    # zeroed elements are dropped either way, so values are unaffected.
    g = np.abs(x_flat.reshape(-1, 4))
    m1 = np.maximum(g[:, 0], g[:, 1]); n1 = np.minimum(g[:, 0], g[:, 1])
    m2 = np.maximum(g[:, 2], g[:, 3]); n2 = np.minimum(g[:, 2], g[:, 3])
    thr = np.maximum(np.minimum(m1, m2), np.maximum(n1, n2))
    third = np.minimum(np.minimum(m1, m2), np.maximum(n1, n2))
    tied = np.flatnonzero(thr == third)
    if len(tied) == 0:
        return x_flat
    x_flat = x_flat.copy()
    gv = x_flat.reshape(-1, 4)
    for t in tied:
        row = gv[t]
        order = np.argsort(-np.abs(row), kind="stable")
        row[order[2:]] = 0.0
    return x_flat


def _prep_inputs(x: np.ndarray, weight: np.ndarray) -> list:
    import ml_dtypes
    x_flat = np.ascontiguousarray(x.reshape(TOK_TOTAL, D), dtype=np.float32)
    x_flat = _fix_ties(x_flat)
    wt = np.ascontiguousarray(weight.T.astype(ml_dtypes.bfloat16))
    return [{"xs": x_flat[c * TOK:(c + 1) * TOK], "wt": wt}
            for c in range(N_CORES)]


def kernel(x: np.ndarray, weight: np.ndarray) -> np.ndarray:
    from concourse.bass_utils import run_bass_kernel_spmd

    nc = _get_compiled()
    in_maps = _prep_inputs(x, weight)
    res = run_bass_kernel_spmd(nc, in_maps, core_ids=list(range(N_CORES)))
    out = np.concatenate([res.results[c]["o"] for c in range(N_CORES)], axis=0)
    return out.reshape(BS, SEQ, OUTF)


# revision 9
# speedup vs baseline: 1.0390x; 1.0390x over previous
# Trainium2 Bass kernel: 2:4 structured activation pruning + Linear.
#
#   out = magnitude_prune_2of4(x.reshape(-1, 4096)) @ weight.T
#
# Sharding: data-parallel over the flattened token dim (16384 tokens ->
# 2048/core across 8 cores); weight replicated (host-transposed + bf16 so
# the contraction dim lands on SBUF partitions). No collectives.
#
# v2 pipeline (PE does ONLY matmuls; transposes moved to the DMA xbar):
#   DMA x (f32) -> DVE |x| pairwise max/min (custom ops, exact f32)
#   -> DVE compact tree -> per-group-of-4 2nd-max threshold (exact f32)
#   -> DVE prune-select writing bf16 -> DMA-xbar transpose (SBUF->SBUF,
#   scalar HWDGE queue) -> PE bf16 matmuls (FWL weight loads) accumulating
#   over 32 d-chunks -> ACT PSUM->SBUF copy -> DMA out (f32).
import numpy as np

N_CORES = 8
BS, SEQ, D = 4, 4096, 4096
OUTF = 1024
TOK_TOTAL = BS * SEQ
TOK = TOK_TOTAL // N_CORES      # 2048 tokens per core
P = 128                         # SBUF partitions
NT = TOK // P                   # 16 token tiles per core
HALF = D // 2                   # 2048: free-dim half width
NCH = D // P                    # 32 d-chunks of 128
NCH_H = NCH // 2                # 16 d-chunks per half

_compiled = None
_custom_ops = None


def _register_custom_dve():
    # Fused DVE ops (registered into the runtime op table; compiled into the
    # per-NEFF DVE table): pairwise abs-max/abs-min, and the pruning select
    # out = |x| >= thr ? x : 0. Halves DVE work vs stock-op sequences.
    global _custom_ops
    if _custom_ops is not None:
        return _custom_ops
    from concourse import dve_ops as D
    from concourse.dve_spec import Spec, Src0, Src1, Zero, maxx, minn, select, lower
    from concourse.dve_uop import DveOpSpec

    def mk(name, body, reference):
        spec = Spec(body=body, reference=reference)
        shas = {}
        for ver in ("v3", "v4"):
            try:
                u = lower(spec, ver=ver)
                shas[ver] = DveOpSpec(name=name, opcode=1, uops=u,
                                      rd1_en=True).sha(ver)
            except Exception:
                if ver == "v3":
                    raise
        return D.DveOp(name=name, spec=spec, subdim=False, uops_sha=shas)

    absa = maxx(Src0, Zero - Src0)
    absb = maxx(Src1, Zero - Src1)
    ops = (
        mk("ABS_MAX2_ANT", maxx(absa, absb),
           lambda in0, in1: np.maximum(np.abs(in0), np.abs(in1))),
        mk("ABS_MIN2_ANT", minn(absa, absb),
           lambda in0, in1: np.minimum(np.abs(in0), np.abs(in1))),
        mk("PRUNE24_ANT", select(maxx(Src0, Zero - Src0) >= Src1, Src0, Zero),
           lambda in0, in1: np.where(np.abs(in0) >= in1, in0, 0.0)),
    )
    for op in ops:
        if op.name not in D._SUB_OPCODE_FOR_NAME:
            D.OPS.append(op)
            D.CUSTOM_DVE_SPECS[op.name] = op.spec
            D._SUB_OPCODE_FOR_NAME[op.name] = (
                D._CUSTOM_DVE_ROW_BASE + len(D._SUB_OPCODE_FOR_NAME))
    _custom_ops = ops
    return ops


def _build():
    import concourse.tile as tile
    import concourse.mybir as mybir
    from concourse import bacc

    ABS_MAX2, ABS_MIN2, PRUNE24 = _register_custom_dve()
    f32 = mybir.dt.float32
    bf16 = mybir.dt.bfloat16
    Alu = mybir.AluOpType

    nc = bacc.Bacc("TRN2", target_bir_lowering=False, debug=False,
                   num_devices=N_CORES)
    xs_ap = nc.dram_tensor("xs", [TOK, D], f32, kind="ExternalInput").ap()
    wt_ap = nc.dram_tensor("wt", [D, OUTF], bf16, kind="ExternalInput").ap()
    o_ap = nc.dram_tensor("o", [TOK, OUTF], f32, kind="ExternalOutput").ap()

    with tile.TileContext(nc) as tc:
        with tc.tile_pool(name="wpool", bufs=1) as wpool, \
             tc.tile_pool(name="xin", bufs=2) as xin, \
             tc.tile_pool(name="mwork", bufs=2) as mwork, \
             tc.tile_pool(name="twork", bufs=2) as twork, \
             tc.tile_pool(name="spool", bufs=2) as spool, \
             tc.tile_pool(name="xtp", bufs=2) as xtp, \
             tc.tile_pool(name="outp", bufs=2) as outp, \
             tc.tile_pool(name="pso", bufs=4, space="PSUM") as pso:

            # weight.T resident in SBUF: [d-in-chunk partitions, chunk, outf],
            # bf16, split in two tiles so the first 16 chunk-matmuls only
            # wait on the first 4MB.  One 3D DMA each, dispatched first on
            # the scalar HWDGE queue (fire-and-forget; the sync queue ring
            # streams x loads in parallel).
            w_halves = []
            for hw in range(2):
                w_h = wpool.tile([P, NCH_H, OUTF], bf16, tag=f"w{hw}")
                src = wt_ap[hw * (D // 2):(hw + 1) * (D // 2), :]
                nc.scalar.dma_start(
                    out=w_h, in_=src.rearrange("(c p) o -> p c o", p=P))
                w_halves.append(w_h)

            def process_span(i, xT, h):
                # prune x[i-tile, h-half] and deposit the transposed bf16
                # chunks into xT[:, h*16:(h+1)*16, :] via the DMA xbar.
                xh = xin.tile([P, HALF], f32, tag="xh")
                nc.sync.dma_start(out=xh, in_=xs_ap[i * P:(i + 1) * P,
                                                    h * HALF:(h + 1) * HALF])
                # pairwise tree: thr = 2nd-largest |x| per group of 4
                x2 = xh.rearrange("p (g two) -> p g two", two=2)
                mx = mwork.tile([P, HALF // 2], f32, tag="mx")
                mn = mwork.tile([P, HALF // 2], f32, tag="mn")
                nc.vector._custom_dve(ABS_MAX2, out=mx,
                                      in0=x2[:, :, 0], in1=x2[:, :, 1])
                nc.vector._custom_dve(ABS_MIN2, out=mn,
                                      in0=x2[:, :, 0], in1=x2[:, :, 1])
                # compact: 2nd-max = max(min of pair-maxes, max of pair-mins)
                mx2 = mx.rearrange("p (g two) -> p g two", two=2)
                mn2 = mn.rearrange("p (g two) -> p g two", two=2)
                mm = twork.tile([P, HALF // 4], f32, tag="mm")
                nm = twork.tile([P, HALF // 4], f32, tag="nm")
                nc.vector.tensor_tensor(mm, mx2[:, :, 0], mx2[:, :, 1], Alu.min)
                nc.vector.tensor_tensor(nm, mn2[:, :, 0], mn2[:, :, 1], Alu.max)
                thr = mm
                nc.vector.tensor_tensor(thr, mm, nm, Alu.max)
                # prune: xspr = |x| >= thr ? x : 0, cast to bf16 on write
                thr_b = thr.unsqueeze(2).broadcast_to([P, HALF // 4, 4])
                xspr = spool.tile([P, HALF], bf16, tag="xspr")
                nc.vector._custom_dve(
                    PRUNE24,
                    out=xspr.rearrange("p (g four) -> p g four", four=4),
                    in0=xh.rearrange("p (g four) -> p g four", four=4),
                    in1=thr_b)
                # SBUF->SBUF 128x128 transposes via the DMA xbar:
                # out[p, c, t] = xspr[t, 128c + p].  On the sync queue with
                # the x loads (same producer-side dependency cadence) so
                # they never queue behind PSUM-copy semaphore waits.
                nc.sync.dma_start(out=xT[h], in_=xspr, transpose=True)

            for i in range(NT):
                # bf16 transposed pruned activations, [d, chunk, tok];
                # one tile per half so matmuls of half 0 can start while
                # half 1 is still being pruned/transposed.
                xT0 = xtp.tile([P, NCH_H, P], bf16, tag="xt0")
                xT1 = xtp.tile([P, NCH_H, P], bf16, tag="xt1")
                xT = [xT0, xT1]
                for h in range(2):
                    process_span(i, xT, h)
                # matmul: psum[tok, outf-half] += xT[h][c].T @ wT[h][c]
                for n in range(2):
                    pout = pso.tile([P, OUTF // 2], f32)
                    for h in range(2):
                        for c in range(NCH_H):
                            nc.tensor.matmul(
                                pout,
                                xT[h][:, c, :],
                                w_halves[h][:, c, n * 512:(n + 1) * 512],
                                start=(h == 0 and c == 0),
                                stop=(h == 1 and c == NCH_H - 1))
                    osb = outp.tile([P, OUTF // 2], f32)
                    nc.scalar.copy(osb, pout)
                    nc.gpsimd.dma_start(
                        out=o_ap[i * P:(i + 1) * P, n * 512:(n + 1) * 512],
                        in_=osb)
    nc.compile()
    return nc


def _get_compiled():
    global _compiled
    if _compiled is None:
        _compiled = _build()
    return _compiled


def _fix_ties(x_flat):
    # The device keeps elements with |x| >= (2nd-largest |x| of the group).
    # On an exact fp32 tie |2nd|==|3rd| that keeps 3 elements, while the
    # reference (top_k, stable) keeps the lower-indexed 2. Pre-zero the
    # reference-dropped elements of tied groups so the device agrees; the
    # zeroed elements are dropped either way, so values are unaffected.
    g = np.abs(x_flat.reshape(-1, 4))
    m1 = np.maximum(g[:, 0], g[:, 1]); n1 = np.minimum(g[:, 0], g[:, 1])
    m2 = np.maximum(g[:, 2], g[:, 3]); n2 = np.minimum(g[:, 2], g[:, 3])
    thr = np.maximum(np.minimum(m1, m2), np.maximum(n1, n2))
    third = np.minimum(np.minimum(m1, m2), np.maximum(n1, n2))
    tied = np.flatnonzero(thr == third)
    if len(tied) == 0:
        return x_flat
    x_flat = x_flat.copy()
    gv = x_flat.reshape(-1, 4)
    for t in tied:
        row = gv[t]
        order = np.argsort(-np.abs(row), kind="stable")
        row[order[2:]] = 0.0
    return x_flat


def _prep_inputs(x: np.ndarray, weight: np.ndarray) -> list:
    import ml_dtypes
    x_flat = np.ascontiguousarray(x.reshape(TOK_TOTAL, D), dtype=np.float32)
    x_flat = _fix_ties(x_flat)
    wt = np.ascontiguousarray(weight.T.astype(ml_dtypes.bfloat16))
    return [{"xs": x_flat[c * TOK:(c + 1) * TOK], "wt": wt}
            for c in range(N_CORES)]


def kernel(x: np.ndarray, weight: np.ndarray) -> np.ndarray:
    from concourse.bass_utils import run_bass_kernel_spmd

    nc = _get_compiled()
    in_maps = _prep_inputs(x, weight)
    res = run_bass_kernel_spmd(nc, in_maps, core_ids=list(range(N_CORES)))
    out = np.concatenate([res.results[c]["o"] for c in range(N_CORES)], axis=0)
    return out.reshape(BS, SEQ, OUTF)
